# revision 15
# baseline (speedup 1.0000x reference)
"""Center-loss kernel for Trainium2 (Bass/Tile), 8-core data-parallel.

loss = 2 - 2 * (sum_i feature[i, label[i]] / 64) / 8192

Sharding: batch dim (8192 rows) split evenly across 8 NeuronCores.
Each core gathers its 1024 picked elements straight out of DRAM with a
single indirect DMA (no full-matrix read), reduces them to one scalar
partial sum on-device, and the host combines the 8 partials.
"""

import sys

if "/opt/trn_rl_repo" not in sys.path:
    sys.path.insert(0, "/opt/trn_rl_repo")

import numpy as np

import concourse.bass as bass
import concourse.bacc as bacc
import concourse.bass_isa as bass_isa
import concourse.tile as tile
from concourse import mybir
from concourse import bass_utils
import concourse.tile_sem_assignment as _tsa

# Keep every Pool-issued DMA on one DMA-SW semaphore lane. The kernel's
# dataflow is a straight chain, and walrus codegen on this toolchain
# rejects instructions (incl. the kernel-tail drain) that need more than
# a few distinct sem waits ("Too many sync wait commands").
_tsa.NUM_SWDGE_GLOBAL_SEMS = 1

N = 8192          # batch rows
C = 10000         # num classes (feature columns)
N_CORES = 8
R = N // N_CORES  # rows per core
P = 128           # SBUF partitions
K = R // P        # picked elements per partition
SCALE = 64.0

_NC_CACHE = None


def _build_nc() -> bass.Bass:
    global _NC_CACHE
    if _NC_CACHE is not None:
        return _NC_CACHE

    # Bacc (not raw Bass): its compile() legalizes multi-wait instructions
    # into separate event-semaphore waits (HW allows 1 sync wait per inst).
    nc = bacc.Bacc(trn_type="TRN2")
    feat = nc.dram_tensor("feature", [R, C], mybir.dt.float32, kind="ExternalInput")
    # labpack[0] = labels, labpack[1] = arange(R)*C (constant row offsets).
    # One DMA loads both, so the index add has a single sem wait — the TT
    # ISA slot on TRN2 can't encode two sync waits.
    lab = nc.dram_tensor("labpack", [2, R], mybir.dt.int32, kind="ExternalInput")
    out = nc.dram_tensor("out", [1, 1], mybir.dt.float32, kind="ExternalOutput")

    # Only three semaphores are in play (DMASW0 for all Pool-issued DMAs,
    # Pool for the index add, DVE for the reduces) so no instruction —
    # including the kernel-tail drain — exceeds the ISA sync-wait slots.
    with tile.TileContext(nc) as tc:
        with tc.tile_pool(name="sbuf", bufs=1) as sp:
            # Local row r = p*K + j lives at tile position [p, :, j].
            lp_tile = sp.tile([P, 2, K], mybir.dt.int32)
            nc.gpsimd.dma_start(
                out=lp_tile[:], in_=lab[:].rearrange("t (p k) -> p t k", p=P)
            )

            # Flat element index of feature[r, label[r]] = r*C + label[r].
            idx = sp.tile([P, K], mybir.dt.int32)
            nc.gpsimd.tensor_tensor(
                out=idx[:],
                in0=lp_tile[:, 0, :],
                in1=lp_tile[:, 1, :],
                op=mybir.AluOpType.add,
            )

            gathered = sp.tile([P, K], mybir.dt.float32)
            nc.gpsimd.indirect_dma_start(
                out=gathered[:],
                out_offset=None,
                in_=feat[:],
                # axis=1 -> coef = prod(shape[2:]) = 1: indices are flat
                # element offsets into the contiguous [R, C] block.
                in_offset=bass.IndirectOffsetOnAxis(ap=idx[:], axis=1),
            )

            # One GPSIMD reduce over partitions + free dims -> scalar. Keeps
            # the whole chain on one engine + one DMA lane.
            res = sp.tile([1, 1], mybir.dt.float32)
            nc.gpsimd.tensor_reduce(
                out=res[:],
                in_=gathered[:],
                axis=mybir.AxisListType.XYZWC,
                op=mybir.AluOpType.add,
            )
            nc.gpsimd.dma_start(out=out[:], in_=res[:])

    nc.finalize()
    _NC_CACHE = nc
    return nc


def _run(feature: np.ndarray, label: np.ndarray, **spmd_kwargs):
    nc = _build_nc()
    feature = np.ascontiguousarray(feature, dtype=np.float32)
    lab32 = np.ascontiguousarray(np.asarray(label).astype(np.int32))
    assert feature.shape == (N, C), feature.shape
    assert lab32.shape == (N,), lab32.shape

    row_off = (np.arange(R, dtype=np.int32) * C).astype(np.int32)
    in_maps = [
        {
            "feature": feature[c * R : (c + 1) * R],
            "labpack": np.stack([lab32[c * R : (c + 1) * R], row_off]),
        }
        for c in range(N_CORES)
    ]
    res = bass_utils.run_bass_kernel_spmd(
        nc, in_maps, core_ids=list(range(N_CORES)), **spmd_kwargs
    )
    partials = np.array(
        [m["out"].reshape(()) for m in res.results], dtype=np.float32
    )
    total = np.float32(partials.sum(dtype=np.float32))
    loss = np.float32(2.0) - np.float32(2.0) * (total / np.float32(SCALE)) / np.float32(N)
    return np.asarray(loss, dtype=np.float32), res


def kernel(feature: np.ndarray, label: np.ndarray) -> np.ndarray:
    loss, _ = _run(feature, label)
    return loss
